# revision 1
# baseline (speedup 1.0000x reference)
"""Autoformer forward on 8 Trainium2 NeuronCores.

Strategy (per spec sharding_hint): data-parallel over batch B=16 -> 2
batches per core. All large GEMMs (QKV/O projections, FFN with fused
on-device gelu) run on the NeuronCores via Bass/Tile programs, executed
SPMD through the bass2jax PJRT path (the axon redirect target of
bass_utils.run_bass_kernel_spmd) so each distinct program compiles once
per process. Activations travel in transposed (D, tokens) layout so the
TensorEngine needs no on-device transposes: Y^T = W^T-free matmul with
lhsT = W tile (K x N), rhs = X^T (K x M).

Cheap irregular ops (FFT autocorrelation, top-k, series decomposition,
layernorm, tiny C=7 convs/projections) run on host numpy - together they
are <1% of model FLOPs.
"""

import math
import time
import numpy as np
import jax
from jax.sharding import Mesh, NamedSharding, PartitionSpec as P

import concourse.bass as bass  # noqa: F401  (registers bass machinery)
import concourse.mybir as mybir
import concourse.tile as tile
from concourse.bass2jax import bass_jit, bass_shard_map

# ---- static model config (matches reference) ----
B, L, C = 16, 1024, 7
D, H, DFF = 512, 8, 2048
MA, FACTOR = 25, 1
NCORES = 8
BS = B // NCORES          # batches per core
M = BS * L                # 2048 token rows per core
F32 = mybir.dt.float32
GELU = mybir.ActivationFunctionType.Gelu_apprx_tanh

DEVICE_NS = 0.0           # accumulated wall-time of device calls (post-warmup)


# ================= device programs =================

def _gemm_tiles(nc, tc, xt, w, yt, gelu):
    """yt[0] = (w.T @ xt[0]) with optional gelu. xt: (1,K,M) w: (K,N) yt: (1,N,M)."""
    K, Mm = xt.shape[1], xt.shape[2]
    N = w.shape[1]
    nk = K // 128
    MT = 512
    with tc.tile_pool(name="w", bufs=1) as wpool, \
         tc.tile_pool(name="x", bufs=2) as xpool, \
         tc.tile_pool(name="o", bufs=4) as opool, \
         tc.tile_pool(name="ps", bufs=4, space="PSUM") as pspool:
        wt = []
        for k in range(nk):
            t = wpool.tile([128, N], F32, tag=f"w{k}")
            nc.sync.dma_start(out=t[:], in_=w[k * 128:(k + 1) * 128, :])
            wt.append(t)
        for m0 in range(0, Mm, MT):
            xts = []
            for k in range(nk):
                t = xpool.tile([128, MT], F32, tag=f"x{k}")
                nc.sync.dma_start(out=t[:], in_=xt[0, k * 128:(k + 1) * 128, m0:m0 + MT])
                xts.append(t)
            for n0 in range(0, N, 128):
                ps = pspool.tile([128, MT], F32, tag="ps")
                for k in range(nk):
                    nc.tensor.matmul(ps[:], wt[k][:, n0:n0 + 128], xts[k][:],
                                     start=(k == 0), stop=(k == nk - 1))
                ot = opool.tile([128, MT], F32, tag="o")
                if gelu:
                    nc.scalar.activation(ot[:], ps[:], GELU)
                else:
                    nc.vector.tensor_copy(out=ot[:], in_=ps[:])
                nc.sync.dma_start(out=yt[0, n0:n0 + 128, m0:m0 + MT], in_=ot[:])


@bass_jit
def _gemm_prog(nc, xt, w):
    N = w.shape[1]
    yt = nc.dram_tensor("yt", [1, N, xt.shape[2]], F32, kind="ExternalOutput")
    with tile.TileContext(nc) as tc:
        _gemm_tiles(nc, tc, xt, w, yt, gelu=False)
    return (yt,)


@bass_jit
def _ffn_prog(nc, xt, w1, w2):
    """yt = w2.T @ gelu(w1.T @ xt): fused two-GEMM FFN, gelu on ScalarE."""
    K, Mm = xt.shape[1], xt.shape[2]
    F = w1.shape[1]
    Dd = w2.shape[1]
    yt = nc.dram_tensor("yt", [1, Dd, Mm], F32, kind="ExternalOutput")
    MT = 512
    nk, nf, nd = K // 128, F // 128, Dd // 128
    with tile.TileContext(nc) as tc:
        with tc.tile_pool(name="w1", bufs=1) as w1pool, \
             tc.tile_pool(name="w2", bufs=1) as w2pool, \
             tc.tile_pool(name="x", bufs=2) as xpool, \
             tc.tile_pool(name="h", bufs=2) as hpool, \
             tc.tile_pool(name="o", bufs=4) as opool, \
             tc.tile_pool(name="ps", bufs=4, space="PSUM") as pspool:
            w1t = []
            for k in range(nk):
                t = w1pool.tile([128, F], F32, tag=f"w1_{k}")
                nc.sync.dma_start(out=t[:], in_=w1[k * 128:(k + 1) * 128, :])
                w1t.append(t)
            w2t = []
            for f in range(nf):
                t = w2pool.tile([128, Dd], F32, tag=f"w2_{f}")
                nc.sync.dma_start(out=t[:], in_=w2[f * 128:(f + 1) * 128, :])
                w2t.append(t)
            for m0 in range(0, Mm, MT):
                xts = []
                for k in range(nk):
                    t = xpool.tile([128, MT], F32, tag=f"x{k}")
                    nc.sync.dma_start(out=t[:], in_=xt[0, k * 128:(k + 1) * 128, m0:m0 + MT])
                    xts.append(t)
                hts = []
                for f0 in range(nf):
                    ps = pspool.tile([128, MT], F32, tag="ps")
                    for k in range(nk):
                        nc.tensor.matmul(ps[:], w1t[k][:, f0 * 128:(f0 + 1) * 128], xts[k][:],
                                         start=(k == 0), stop=(k == nk - 1))
                    ht = hpool.tile([128, MT], F32, tag=f"h{f0}")
                    nc.scalar.activation(ht[:], ps[:], GELU)
                    hts.append(ht)
                for d0 in range(nd):
                    ps = pspool.tile([128, MT], F32, tag="ps")
                    for f in range(nf):
                        nc.tensor.matmul(ps[:], w2t[f][:, d0 * 128:(d0 + 1) * 128], hts[f][:],
                                         start=(f == 0), stop=(f == nf - 1))
                    ot = opool.tile([128, MT], F32, tag="o")
                    nc.vector.tensor_copy(out=ot[:], in_=ps[:])
                    nc.sync.dma_start(out=yt[0, d0 * 128:(d0 + 1) * 128, m0:m0 + MT], in_=ot[:])
    return (yt,)


# ================= host-side SPMD plumbing =================

_mesh = None
_progs = {}


def _get_mesh():
    global _mesh
    if _mesh is None:
        _mesh = Mesh(np.array(jax.devices()[:NCORES]), ("c",))
    return _mesh


def _sharded(name, fn, nin):
    if name not in _progs:
        mesh = _get_mesh()
        specs = (P("c"),) + (P(),) * (nin - 1)
        _progs[name] = bass_shard_map(fn, mesh=mesh, in_specs=specs, out_specs=P("c"))
    return _progs[name]


def _to_xt(x):
    """(B, L, K) -> sharded-transposed (NCORES, K, M) contiguous f32."""
    K = x.shape[2]
    return np.ascontiguousarray(x.reshape(NCORES, M, K).transpose(0, 2, 1),
                                dtype=np.float32)


def _from_yt(out, N):
    """(NCORES, N, M) -> (B, L, N)."""
    return np.asarray(out).transpose(0, 2, 1).reshape(B, L, N)


def _run(prog, *arrs):
    global DEVICE_NS
    mesh = _get_mesh()
    shardings = [NamedSharding(mesh, P("c"))] + \
                [NamedSharding(mesh, P())] * (len(arrs) - 1)
    js = [jax.device_put(a, s) for a, s in zip(arrs, shardings)]
    t0 = time.perf_counter()
    (out,) = prog(*js)
    out.block_until_ready()
    DEVICE_NS += (time.perf_counter() - t0) * 1e9
    return out


def _dev_gemm(x, w):
    """(B,L,K) @ (K,N) -> (B,L,N) on the 8 cores, batch-parallel."""
    N = w.shape[1]
    prog = _sharded("gemm", _gemm_prog, 2)
    out = _run(prog, _to_xt(x), np.ascontiguousarray(w, np.float32))
    return _from_yt(out, N)


def _dev_ffn(x, w1, w2):
    prog = _sharded("ffn", _ffn_prog, 3)
    out = _run(prog, _to_xt(x),
               np.ascontiguousarray(w1, np.float32),
               np.ascontiguousarray(w2, np.float32))
    return _from_yt(out, w2.shape[1])


# ================= host numpy model glue =================

def _series_decomp(x):
    pad = (MA - 1) // 2
    front = np.repeat(x[:, :1], pad, axis=1)
    end = np.repeat(x[:, -1:], pad, axis=1)
    xp = np.concatenate([front, x, end], axis=1)
    cs = np.cumsum(xp.astype(np.float64), axis=1)
    cs = np.concatenate([np.zeros_like(cs[:, :1]), cs], axis=1)
    trend = ((cs[:, MA:] - cs[:, :-MA]) / MA).astype(np.float32)
    return (x - trend).astype(np.float32), trend


def _circ_conv1d(x, w):
    p = (w.shape[0] - 1) // 2
    xp = np.concatenate([x[:, -p:], x, x[:, :p]], axis=1)
    out = xp[:, 0:x.shape[1]] @ w[0]
    for j in range(1, w.shape[0]):
        out += xp[:, j:j + x.shape[1]] @ w[j]
    return out.astype(np.float32)


def _my_layernorm(x, g, b):
    mu = x.mean(-1, keepdims=True)
    var = ((x - mu) ** 2).mean(-1, keepdims=True)
    xh = (x - mu) / np.sqrt(var + 1e-5) * g + b
    return (xh - xh.mean(axis=1, keepdims=True)).astype(np.float32)


def _autocorr(q, k, v):
    """q,k,v: (B,L,H,E) -> (B,L,H,E); FFT correlation + top-k delay aggregation."""
    Lq = q.shape[1]
    qT = q.transpose(0, 2, 3, 1)
    kT = k.transpose(0, 2, 3, 1)
    vT = v.transpose(0, 2, 3, 1)
    qf = np.fft.rfft(qT, axis=-1)
    kf = np.fft.rfft(kT, axis=-1)
    corr = np.fft.irfft(qf * np.conj(kf), n=Lq, axis=-1)
    mean_corr = corr.mean(axis=(1, 2))                       # (B, L)
    tk = int(FACTOR * math.log(Lq))
    idx = np.argpartition(-mean_corr, tk - 1, axis=-1)[:, :tk]
    wts = np.take_along_axis(mean_corr, idx, axis=-1).astype(np.float32)
    e = np.exp(wts - wts.max(-1, keepdims=True))
    sm = e / e.sum(-1, keepdims=True)
    agg = np.zeros(vT.shape, np.float32)
    for b in range(vT.shape[0]):
        for j in range(tk):
            agg[b] += sm[b, j] * np.roll(vT[b], -int(idx[b, j]), axis=-1)
    return agg.transpose(0, 3, 1, 2)


_ZPAD = None


def _ac_layer(xq, xkv, p, cross=False):
    global _ZPAD
    if _ZPAD is None:
        _ZPAD = np.zeros((D, D), np.float32)
    if not cross:
        w = np.concatenate([p["wq"], p["wk"], p["wv"]], axis=1)
        qkv = _dev_gemm(xq, w)
        q, k, v = qkv[..., :D], qkv[..., D:2 * D], qkv[..., 2 * D:]
    else:
        q = _dev_gemm(xq, np.concatenate([p["wq"], _ZPAD, _ZPAD], axis=1))[..., :D]
        kv = _dev_gemm(xkv, np.concatenate([p["wk"], p["wv"], _ZPAD], axis=1))
        k, v = kv[..., :D], kv[..., D:2 * D]
    q = q + p["bq"]
    k = k + p["bk"]
    v = v + p["bv"]
    Bq, Lq = xq.shape[0], xq.shape[1]
    out = _autocorr(q.reshape(Bq, Lq, H, -1),
                    k.reshape(Bq, Lq, H, -1),
                    v.reshape(Bq, Lq, H, -1)).reshape(Bq, Lq, D)
    o = _dev_gemm(out, np.concatenate([p["wo"], _ZPAD, _ZPAD], axis=1))[..., :D]
    return (o + p["bo"]).astype(np.float32)


def _encoder_layer(x, p):
    x = x + _ac_layer(x, x, p["attn"])
    x, _ = _series_decomp(x)
    y = _dev_ffn(x, p["w1"], p["w2"])
    x, _ = _series_decomp((x + y).astype(np.float32))
    return x


def _decoder_layer(x, crossv, p):
    x = x + _ac_layer(x, x, p["self"])
    x, t1 = _series_decomp(x)
    x = x + _ac_layer(x, crossv, p["cross"], cross=True)
    x, t2 = _series_decomp(x)
    y = _dev_ffn(x, p["w1"], p["w2"])
    x, t3 = _series_decomp((x + y).astype(np.float32))
    residual_trend = _circ_conv1d(t1 + t2 + t3, p["trend_proj"])
    return x, residual_trend


def kernel(params, x_enc):
    params = jax.tree.map(lambda a: np.asarray(a, np.float32), params)
    x_enc = np.asarray(x_enc, np.float32)

    seasonal_init, trend_init = _series_decomp(x_enc)
    enc = _circ_conv1d(x_enc, params["enc_emb"])
    for lp in params["enc_layers"]:
        enc = _encoder_layer(enc, lp)
    enc = _my_layernorm(enc, params["enc_norm_g"], params["enc_norm_b"])

    dec = _circ_conv1d(seasonal_init, params["dec_emb"])
    trend = trend_init
    for lp in params["dec_layers"]:
        dec, rt = _decoder_layer(dec, enc, lp)
        trend = trend + rt
    dec = _my_layernorm(dec, params["dec_norm_g"], params["dec_norm_b"])
    dec = dec @ params["dec_proj_w"] + params["dec_proj_b"]
    dec_out = (dec + trend).astype(np.float32)
    pred = dec_out.reshape(B, -1) @ params["pred_w"] + params["pred_b"]
    return pred.astype(np.float32)


# revision 5
# speedup vs baseline: 1.1300x; 1.1300x over previous
"""Autoformer forward on 8 Trainium2 NeuronCores.

Strategy (per spec sharding_hint): data-parallel over batch B=16 -> 2
batches per core. All large GEMMs (QKV/O projections, FFN with fused
on-device gelu) run on the NeuronCores via Bass/Tile programs, executed
SPMD through the bass2jax PJRT path (the axon redirect target of
bass_utils.run_bass_kernel_spmd) so each distinct program compiles once
per process. Activations travel in transposed (D, tokens) layout so the
TensorEngine needs no on-device transposes: Y^T = W^T-free matmul with
lhsT = W tile (K x N), rhs = X^T (K x M).

Cheap irregular ops (FFT autocorrelation, top-k, series decomposition,
layernorm, tiny C=7 convs/projections) run on host numpy - together they
are <1% of model FLOPs.
"""

import math
import time
import numpy as np
import scipy.fft as sfft
import jax
from jax.sharding import Mesh, NamedSharding, PartitionSpec as P

import concourse.bass as bass  # noqa: F401  (registers bass machinery)
import concourse.mybir as mybir
import concourse.tile as tile
from concourse.bass2jax import bass_jit, bass_shard_map

# ---- static model config (matches reference) ----
B, L, C = 16, 1024, 7
D, H, DFF = 512, 8, 2048
MA, FACTOR = 25, 1
NCORES = 8
BS = B // NCORES          # batches per core
M = BS * L                # 2048 token rows per core
F32 = mybir.dt.float32
GELU = mybir.ActivationFunctionType.Gelu_apprx_tanh

DEVICE_NS = 0.0           # accumulated wall-time of device calls (post-warmup)


# ================= device programs =================

def _gemm_tiles(nc, tc, xt, w, yt, gelu):
    """yt[0] = (w.T @ xt[0]) with optional gelu. xt: (1,K,M) w: (K,N) yt: (1,N,M)."""
    K, Mm = xt.shape[1], xt.shape[2]
    N = w.shape[1]
    nk = K // 128
    MT = 512
    with tc.tile_pool(name="w", bufs=1) as wpool, \
         tc.tile_pool(name="x", bufs=2) as xpool, \
         tc.tile_pool(name="o", bufs=4) as opool, \
         tc.tile_pool(name="ps", bufs=4, space="PSUM") as pspool:
        wt = []
        for k in range(nk):
            t = wpool.tile([128, N], F32, tag=f"w{k}")
            nc.sync.dma_start(out=t[:], in_=w[k * 128:(k + 1) * 128, :])
            wt.append(t)
        for m0 in range(0, Mm, MT):
            xts = []
            for k in range(nk):
                t = xpool.tile([128, MT], F32, tag=f"x{k}")
                nc.sync.dma_start(out=t[:], in_=xt[0, k * 128:(k + 1) * 128, m0:m0 + MT])
                xts.append(t)
            for n0 in range(0, N, 128):
                ps = pspool.tile([128, MT], F32, tag="ps")
                for k in range(nk):
                    nc.tensor.matmul(ps[:], wt[k][:, n0:n0 + 128], xts[k][:],
                                     start=(k == 0), stop=(k == nk - 1))
                ot = opool.tile([128, MT], F32, tag="o")
                if gelu:
                    nc.scalar.activation(ot[:], ps[:], GELU)
                else:
                    nc.vector.tensor_copy(out=ot[:], in_=ps[:])
                nc.sync.dma_start(out=yt[0, n0:n0 + 128, m0:m0 + MT], in_=ot[:])


@bass_jit
def _gemm_prog(nc, xt, w):
    N = w.shape[1]
    yt = nc.dram_tensor("yt", [1, N, xt.shape[2]], F32, kind="ExternalOutput")
    with tile.TileContext(nc) as tc:
        _gemm_tiles(nc, tc, xt, w, yt, gelu=False)
    return (yt,)


@bass_jit
def _ffn_prog(nc, xt, w1, w2):
    """yt = w2.T @ gelu(w1.T @ xt): fused two-GEMM FFN, gelu on ScalarE."""
    K, Mm = xt.shape[1], xt.shape[2]
    F = w1.shape[1]
    Dd = w2.shape[1]
    yt = nc.dram_tensor("yt", [1, Dd, Mm], F32, kind="ExternalOutput")
    MT = 512
    nk, nf, nd = K // 128, F // 128, Dd // 128
    with tile.TileContext(nc) as tc:
        with tc.tile_pool(name="w1", bufs=1) as w1pool, \
             tc.tile_pool(name="w2", bufs=1) as w2pool, \
             tc.tile_pool(name="x", bufs=2) as xpool, \
             tc.tile_pool(name="h", bufs=2) as hpool, \
             tc.tile_pool(name="o", bufs=4) as opool, \
             tc.tile_pool(name="ps", bufs=4, space="PSUM") as pspool:
            w1t = []
            for k in range(nk):
                t = w1pool.tile([128, F], F32, tag=f"w1_{k}")
                nc.sync.dma_start(out=t[:], in_=w1[k * 128:(k + 1) * 128, :])
                w1t.append(t)
            w2t = []
            for f in range(nf):
                t = w2pool.tile([128, Dd], F32, tag=f"w2_{f}")
                nc.sync.dma_start(out=t[:], in_=w2[f * 128:(f + 1) * 128, :])
                w2t.append(t)
            for m0 in range(0, Mm, MT):
                xts = []
                for k in range(nk):
                    t = xpool.tile([128, MT], F32, tag=f"x{k}")
                    nc.sync.dma_start(out=t[:], in_=xt[0, k * 128:(k + 1) * 128, m0:m0 + MT])
                    xts.append(t)
                hts = []
                for f0 in range(nf):
                    ps = pspool.tile([128, MT], F32, tag="ps")
                    for k in range(nk):
                        nc.tensor.matmul(ps[:], w1t[k][:, f0 * 128:(f0 + 1) * 128], xts[k][:],
                                         start=(k == 0), stop=(k == nk - 1))
                    ht = hpool.tile([128, MT], F32, tag=f"h{f0}")
                    nc.scalar.activation(ht[:], ps[:], GELU)
                    hts.append(ht)
                for d0 in range(nd):
                    ps = pspool.tile([128, MT], F32, tag="ps")
                    for f in range(nf):
                        nc.tensor.matmul(ps[:], w2t[f][:, d0 * 128:(d0 + 1) * 128], hts[f][:],
                                         start=(f == 0), stop=(f == nf - 1))
                    ot = opool.tile([128, MT], F32, tag="o")
                    nc.vector.tensor_copy(out=ot[:], in_=ps[:])
                    nc.sync.dma_start(out=yt[0, d0 * 128:(d0 + 1) * 128, m0:m0 + MT], in_=ot[:])
    return (yt,)


# ================= host-side SPMD plumbing =================

_mesh = None
_progs = {}


def _get_mesh():
    global _mesh
    if _mesh is None:
        _mesh = Mesh(np.array(jax.devices()[:NCORES]), ("c",))
    return _mesh


def _sharded(name, fn, nin):
    if name not in _progs:
        mesh = _get_mesh()
        specs = (P("c"),) + (P(),) * (nin - 1)
        _progs[name] = bass_shard_map(fn, mesh=mesh, in_specs=specs, out_specs=P("c"))
    return _progs[name]


def _to_xt(x):
    """(B, L, K) -> sharded-transposed (NCORES, K, M) contiguous f32."""
    K = x.shape[2]
    return np.ascontiguousarray(x.reshape(NCORES, M, K).transpose(0, 2, 1),
                                dtype=np.float32)


def _from_yt(out, N):
    """(NCORES, N, M) -> (B, L, N)."""
    return np.asarray(out).transpose(0, 2, 1).reshape(B, L, N)


def _run(prog, *arrs):
    global DEVICE_NS
    mesh = _get_mesh()
    shardings = [NamedSharding(mesh, P("c"))] + \
                [NamedSharding(mesh, P())] * (len(arrs) - 1)
    js = [jax.device_put(a, s) for a, s in zip(arrs, shardings)]
    t0 = time.perf_counter()
    (out,) = prog(*js)
    out.block_until_ready()
    DEVICE_NS += (time.perf_counter() - t0) * 1e9
    return out


def _dev_gemm(x, w):
    """(B,L,K) @ (K,N) -> (B,L,N) on the 8 cores, batch-parallel."""
    N = w.shape[1]
    prog = _sharded("gemm", _gemm_prog, 2)
    out = _run(prog, _to_xt(x), np.ascontiguousarray(w, np.float32))
    return _from_yt(out, N)


def _dev_ffn(x, w1, w2):
    prog = _sharded("ffn", _ffn_prog, 3)
    out = _run(prog, _to_xt(x),
               np.ascontiguousarray(w1, np.float32),
               np.ascontiguousarray(w2, np.float32))
    return _from_yt(out, w2.shape[1])


# ================= host numpy model glue =================

def _series_decomp(x):
    pad = (MA - 1) // 2
    front = np.repeat(x[:, :1], pad, axis=1)
    end = np.repeat(x[:, -1:], pad, axis=1)
    xp = np.concatenate([front, x, end], axis=1)
    cs = np.cumsum(xp, axis=1, dtype=np.float32)
    cs = np.concatenate([np.zeros_like(cs[:, :1]), cs], axis=1)
    trend = ((cs[:, MA:] - cs[:, :-MA]) / MA).astype(np.float32)
    return (x - trend).astype(np.float32), trend


def _circ_conv1d(x, w):
    p = (w.shape[0] - 1) // 2
    xp = np.concatenate([x[:, -p:], x, x[:, :p]], axis=1)
    out = xp[:, 0:x.shape[1]] @ w[0]
    for j in range(1, w.shape[0]):
        out += xp[:, j:j + x.shape[1]] @ w[j]
    return out.astype(np.float32)


def _my_layernorm(x, g, b):
    mu = x.mean(-1, keepdims=True)
    var = ((x - mu) ** 2).mean(-1, keepdims=True)
    xh = (x - mu) / np.sqrt(var + 1e-5) * g + b
    return (xh - xh.mean(axis=1, keepdims=True)).astype(np.float32)


def _autocorr(q, k, v):
    """q,k,v: (B,L,H,E) -> (B,L,H,E); FFT correlation + top-k delay aggregation."""
    Lq = q.shape[1]
    qT = q.transpose(0, 2, 3, 1)
    kT = k.transpose(0, 2, 3, 1)
    vT = v.transpose(0, 2, 3, 1)
    qf = sfft.rfft(qT, axis=-1, workers=16)
    kf = sfft.rfft(kT, axis=-1, workers=16)
    corr = sfft.irfft(qf * np.conj(kf), n=Lq, axis=-1, workers=16)
    mean_corr = corr.mean(axis=(1, 2))                       # (B, L)
    tk = int(FACTOR * math.log(Lq))
    idx = np.argpartition(-mean_corr, tk - 1, axis=-1)[:, :tk]
    wts = np.take_along_axis(mean_corr, idx, axis=-1).astype(np.float32)
    e = np.exp(wts - wts.max(-1, keepdims=True))
    sm = e / e.sum(-1, keepdims=True)
    agg = np.zeros(vT.shape, np.float32)
    for b in range(vT.shape[0]):
        for j in range(tk):
            agg[b] += sm[b, j] * np.roll(vT[b], -int(idx[b, j]), axis=-1)
    return agg.transpose(0, 3, 1, 2)


def _addb(x, b):
    return x + b if np.any(b) else x


def _ac_layer(xq, xkv, p, cross=False):
    if not cross:
        w = np.concatenate([p["wq"], p["wk"], p["wv"]], axis=1)
        qkv = _dev_gemm(xq, w)
        q, k, v = qkv[..., :D], qkv[..., D:2 * D], qkv[..., 2 * D:]
    else:
        q = _dev_gemm(xq, p["wq"])
        kv = _dev_gemm(xkv, np.concatenate([p["wk"], p["wv"]], axis=1))
        k, v = kv[..., :D], kv[..., D:]
    q = _addb(q, p["bq"])
    k = _addb(k, p["bk"])
    v = _addb(v, p["bv"])
    Bq, Lq = xq.shape[0], xq.shape[1]
    out = _autocorr(q.reshape(Bq, Lq, H, -1),
                    k.reshape(Bq, Lq, H, -1),
                    v.reshape(Bq, Lq, H, -1)).reshape(Bq, Lq, D)
    o = _dev_gemm(out, p["wo"])
    return _addb(o, p["bo"]).astype(np.float32)


def _encoder_layer(x, p):
    x = x + _ac_layer(x, x, p["attn"])
    x, _ = _series_decomp(x)
    y = _dev_ffn(x, p["w1"], p["w2"])
    x, _ = _series_decomp((x + y).astype(np.float32))
    return x


def _decoder_layer(x, crossv, p):
    x = x + _ac_layer(x, x, p["self"])
    x, t1 = _series_decomp(x)
    x = x + _ac_layer(x, crossv, p["cross"], cross=True)
    x, t2 = _series_decomp(x)
    y = _dev_ffn(x, p["w1"], p["w2"])
    x, t3 = _series_decomp((x + y).astype(np.float32))
    residual_trend = _circ_conv1d(t1 + t2 + t3, p["trend_proj"])
    return x, residual_trend


def kernel(params, x_enc):
    params = jax.tree.map(lambda a: np.asarray(a, np.float32), params)
    x_enc = np.asarray(x_enc, np.float32)

    seasonal_init, trend_init = _series_decomp(x_enc)
    enc = _circ_conv1d(x_enc, params["enc_emb"])
    for lp in params["enc_layers"]:
        enc = _encoder_layer(enc, lp)
    enc = _my_layernorm(enc, params["enc_norm_g"], params["enc_norm_b"])

    dec = _circ_conv1d(seasonal_init, params["dec_emb"])
    trend = trend_init
    for lp in params["dec_layers"]:
        dec, rt = _decoder_layer(dec, enc, lp)
        trend = trend + rt
    dec = _my_layernorm(dec, params["dec_norm_g"], params["dec_norm_b"])
    dec = dec @ params["dec_proj_w"] + params["dec_proj_b"]
    dec_out = (dec + trend).astype(np.float32)
    pred = dec_out.reshape(B, -1) @ params["pred_w"] + params["pred_b"]
    return pred.astype(np.float32)
